# revision 9
# baseline (speedup 1.0000x reference)
"""Causal multi-head self-attention (B=4, S=2048, D=1024, 16 heads) on 8 TRN2 cores.

Sharding: core c -> batch b = c//2, head-half hh = c%2 (8 of 16 heads, 512 of
1024 projection dims).  Each core:
  - projects its batch's q/k/v against its 512 weight columns (the 1/sqrt(D)
    score scale is folded into Wq/bq on the host),
  - runs causal attention for its 8 heads in a scores-transposed layout
    (keys on partitions, queries on the free dim), exp on ScalarE, softmax
    denominators via an appended ones-column in V,
  - applies its 512 rows of Wo (bias bo/2 added via a K=1 ones matmul),
  - pairwise ReduceScatter sums the two half-head partials; each core ends
    with half the sequence rows of its batch's output.
Host concatenates the halves.  All matmuls run as fp32r (full-rate PE) except
scores (bf16 q/k) -- empirically ~1e-4..1e-3 relative error.
"""

import numpy as np

B = 4
S = 2048
DM = 1024
HD = 64
NH = 8            # heads per core
OD = NH * HD      # 512: per-core projection width
NCORES = 8
QCN = S // 512    # 4 query chunks of 512
DBLK = DM // 128  # 8 contraction blocks
OBLK = OD // 128  # 4 output-dim blocks
SBLK = S // 128   # 16 seq strips

_prog = None


def _build_program():
    from contextlib import ExitStack
    from concourse import bacc, mybir
    import concourse.tile as tile

    f32 = mybir.dt.float32
    f32r = mybir.dt.float32r
    bf16 = mybir.dt.bfloat16
    EXP = mybir.ActivationFunctionType.Exp
    IDENT = mybir.ActivationFunctionType.Identity
    COPY = mybir.ActivationFunctionType.Copy

    nc = bacc.Bacc(None, num_devices=NCORES)

    # --- external I/O (per-core shards) ---
    xq_ext = nc.declare_dram_parameter("xq", [DM, S], f32r, isOutput=False)
    xk_ext = nc.declare_dram_parameter("xk", [DM, S], f32r, isOutput=False)
    xv_ext = nc.declare_dram_parameter("xv", [DM, S], f32r, isOutput=False)
    wq_ext = nc.declare_dram_parameter("wq", [DM, OD], f32r, isOutput=False)
    wk_ext = nc.declare_dram_parameter("wk", [DM, OD], f32r, isOutput=False)
    wv_ext = nc.declare_dram_parameter("wv", [DM, OD], f32r, isOutput=False)
    wo_ext = nc.declare_dram_parameter("wo", [OD, DM], f32r, isOutput=False)
    bq_ext = nc.declare_dram_parameter("bq2", [128, OBLK], f32, isOutput=False)
    bk_ext = nc.declare_dram_parameter("bk2", [128, OBLK], f32, isOutput=False)
    bv_ext = nc.declare_dram_parameter("bvr", [1, OD], f32r, isOutput=False)
    bo_ext = nc.declare_dram_parameter("bo2", [1, DM], f32r, isOutput=False)
    mask_ext = nc.declare_dram_parameter("mask", [128, 128], f32, isOutput=False)
    id_ext = nc.declare_dram_parameter("ident", [128, 128], f32, isOutput=False)
    ones_ext = nc.declare_dram_parameter("ones1", [1, 128], f32r, isOutput=False)
    out_ext = nc.declare_dram_parameter("out", [S // 2, DM], f32, isOutput=True)

    with tile.TileContext(nc) as tc, ExitStack() as ctx:
        consts = ctx.enter_context(tc.tile_pool(name="consts", bufs=1))
        persist = ctx.enter_context(tc.tile_pool(name="persist", bufs=1))
        xpool = ctx.enter_context(tc.tile_pool(name="xpool", bufs=10))
        wpool = ctx.enter_context(tc.tile_pool(name="wpool", bufs=10))
        wopool = ctx.enter_context(tc.tile_pool(name="wopool", bufs=1))
        ppool = ctx.enter_context(tc.tile_pool(name="ppool", bufs=4))
        stg = ctx.enter_context(tc.tile_pool(name="stg", bufs=2))
        ps_mm = ctx.enter_context(tc.tile_pool(name="ps_mm", bufs=4, space="PSUM"))
        ps_acc = ctx.enter_context(tc.tile_pool(name="ps_acc", bufs=2, space="PSUM"))
        ps_tr = ctx.enter_context(tc.tile_pool(name="ps_tr", bufs=2, space="PSUM"))
        dram = ctx.enter_context(tc.tile_pool(name="dram", bufs=1, space="DRAM"))

        # --- constants ---
        ident_sb = consts.tile([128, 128], f32, name="ident_sb")
        mask_sb = consts.tile([128, 128], f32, name="mask_sb")
        bq_sb = consts.tile([128, OBLK], f32, name="bq_sb")
        bk_sb = consts.tile([128, OBLK], f32, name="bk_sb")
        bv_sb = consts.tile([1, OD], f32r, name="bv_sb")
        bo_sb = consts.tile([1, DM], f32r, name="bo_sb")
        ones1 = consts.tile([1, 128], f32r, name="ones1")
        ones_col = consts.tile([128, NH, 1], f32, name="ones_col")
        zeros_sb = consts.tile([128, 384], f32, name="zeros_sb")
        nc.sync.dma_start(out=ident_sb, in_=id_ext[:, :])
        nc.sync.dma_start(out=mask_sb, in_=mask_ext[:, :])
        nc.sync.dma_start(out=bq_sb, in_=bq_ext[:, :])
        nc.sync.dma_start(out=bk_sb, in_=bk_ext[:, :])
        nc.sync.dma_start(out=bv_sb, in_=bv_ext[:, :])
        nc.sync.dma_start(out=bo_sb, in_=bo_ext[:, :])
        nc.sync.dma_start(out=ones1, in_=ones_ext[:, :])
        nc.vector.memset(ones_col, 1.0)
        nc.vector.memset(zeros_sb, 0.0)

        # --- persistent activations ---
        qT = [persist.tile([128, S], bf16, name=f"qT{i}") for i in range(OBLK)]
        kT = [persist.tile([128, S], bf16, name=f"kT{i}") for i in range(OBLK)]
        # v_sb[s]: [128, 8 heads * 65]; col 65h+64 is the ones column
        v_sb = [persist.tile([128, NH * (HD + 1)], f32r, name=f"v{s}") for s in range(SBLK)]
        ctxT = [persist.tile([128, S], f32r, name=f"ctxT{i}") for i in range(OBLK)]

        # --- output-projection weights (prefetch early) ---
        wo_sb = [wopool.tile([128, DM], f32r, name=f"wo{i}", tag=f"wo{i}") for i in range(OBLK)]
        for i in range(OBLK):
            nc.sync.dma_start(out=wo_sb[i], in_=wo_ext[i * 128:(i + 1) * 128, :])

        # ones columns of v
        for s in range(SBLK):
            v3 = v_sb[s].rearrange("p (h e) -> p h e", e=HD + 1)
            nc.vector.tensor_copy(out=v3[:, :, HD:HD + 1], in_=ones_col)

        # --- projection weights ---
        def load_w(w_ext, nm):
            tiles = []
            for d in range(DBLK):
                t = wpool.tile([128, OD], f32r, name=f"{nm}{d}", tag="w")
                nc.sync.dma_start(out=t, in_=w_ext[d * 128:(d + 1) * 128, :])
                tiles.append(t)
            return tiles

        # --- q/k projections: qT[oblk][:, s] = (Wq.T @ x.T + bq) ---
        def proj_qk(x_ext, w_ext, bias_sb, dst, nm):
            w_tiles = load_w(w_ext, nm + "w")
            for sc in range(QCN):
                xt = []
                for d in range(DBLK):
                    t = xpool.tile([128, 512], f32r, name=f"{nm}x{sc}_{d}", tag="xt")
                    nc.sync.dma_start(
                        out=t, in_=x_ext[d * 128:(d + 1) * 128, sc * 512:(sc + 1) * 512])
                    xt.append(t)
                for ob in range(OBLK):
                    psum = ps_mm.tile([128, 512], f32, name=f"{nm}ps{sc}_{ob}", tag="mm")
                    for d in range(DBLK):
                        nc.tensor.matmul(
                            psum, w_tiles[d][:, ob * 128:(ob + 1) * 128], xt[d],
                            start=(d == 0), stop=(d == DBLK - 1))
                    nc.scalar.activation(
                        out=dst[ob][:, sc * 512:(sc + 1) * 512], in_=psum,
                        func=IDENT, bias=bias_sb[:, ob:ob + 1], scale=1.0)

        proj_qk(xq_ext, wq_ext, bq_sb, qT, "q")
        proj_qk(xk_ext, wk_ext, bk_sb, kT, "k")

        # --- v projection: v[s, o] = x @ Wv + bv (natural layout) ---
        wv_tiles = load_w(wv_ext, "vw")
        for sc in range(QCN):
            xt = []
            for d in range(DBLK):
                t = xpool.tile([128, 512], f32r, name=f"vx{sc}_{d}", tag="xt")
                nc.sync.dma_start(
                    out=t, in_=xv_ext[d * 128:(d + 1) * 128, sc * 512:(sc + 1) * 512])
                xt.append(t)
            for sl in range(4):
                s = sc * 4 + sl
                psum = ps_mm.tile([128, 512], f32, name=f"vps{s}", tag="mm")
                for d in range(DBLK):
                    nc.tensor.matmul(
                        psum, xt[d][:, sl * 128:(sl + 1) * 128], wv_tiles[d],
                        start=(d == 0), stop=False)
                nc.tensor.matmul(psum, ones1, bv_sb, start=False, stop=True)
                v3 = v_sb[s].rearrange("p (h e) -> p h e", e=HD + 1)
                ps3 = psum.rearrange("p (h e) -> p h e", e=HD)
                nc.vector.tensor_copy(out=v3[:, :, 0:HD], in_=ps3)

        # --- attention + output projection, chunk by chunk ---
        for qc in range(QCN):
            for h in range(NH):
                t, r0 = h // 2, 64 * (h % 2)
                lq = qT[t][r0:r0 + 64, qc * 512:(qc + 1) * 512]
                acc = ps_acc.tile([HD + 1, 512], f32, name=f"acc{qc}_{h}", tag="acc")
                nkb = 4 * qc + 4
                plist = []
                for kb in range(nkb):
                    sps = ps_mm.tile([128, 512], f32, name=f"s{qc}_{h}_{kb}", tag="mm")
                    nc.tensor.matmul(
                        sps, kT[t][r0:r0 + 64, kb * 128:(kb + 1) * 128], lq,
                        start=True, stop=True)
                    p = ppool.tile([128, 512], f32r, name=f"p{qc}_{h}_{kb}", tag="p")
                    m = kb - 4 * qc
                    if m < 0:
                        nc.scalar.activation(out=p, in_=sps, func=EXP)
                    else:
                        if m > 0:
                            nc.vector.tensor_copy(
                                out=p[:, 0:128 * m], in_=zeros_sb[:, 0:128 * m])
                        nc.scalar.activation(
                            out=p[:, 128 * m:512], in_=sps[:, 128 * m:512], func=EXP)
                        nc.vector.tensor_mul(
                            out=p[:, 128 * m:128 * (m + 1)],
                            in0=p[:, 128 * m:128 * (m + 1)], in1=mask_sb)
                    plist.append((kb, p))
                    # lag the accumulation matmul one block behind the scores
                    # matmul so PE isn't blocked on the exp of the newest block
                    if len(plist) >= 2:
                        pkb, pp = plist.pop(0)
                        nc.tensor.matmul(
                            acc, v_sb[pkb][:, 65 * h:65 * h + 65], pp,
                            start=(pkb == 0), stop=(pkb == nkb - 1))
                pkb, pp = plist.pop(0)
                nc.tensor.matmul(
                    acc, v_sb[pkb][:, 65 * h:65 * h + 65], pp,
                    start=(pkb == 0), stop=(pkb == nkb - 1))

                # normalize + transpose into ctxT
                cstg = stg.tile([HD + 1, 512], f32, name=f"cstg{qc}_{h}", tag="cstg")
                nc.vector.tensor_copy(out=cstg, in_=acc)
                for sl in range(4):
                    tr1 = ps_tr.tile([128, HD + 1], f32, name=f"tr1_{qc}_{h}_{sl}", tag="tr")
                    nc.tensor.transpose(
                        tr1, cstg[:, sl * 128:(sl + 1) * 128], ident_sb[0:HD + 1, 0:HD + 1])
                    recip = stg.tile([128, 1], f32, name=f"rc{qc}_{h}_{sl}", tag="recip", bufs=4)
                    nc.vector.reciprocal(out=recip, in_=tr1[:, HD:HD + 1])
                    ctxn = stg.tile([128, HD], f32, name=f"cn{qc}_{h}_{sl}", tag="ctxn", bufs=4)
                    nc.scalar.activation(out=ctxn, in_=tr1[:, 0:HD], func=COPY, scale=recip)
                    tr2 = ps_tr.tile([HD, 128], f32, name=f"tr2_{qc}_{h}_{sl}", tag="tr")
                    nc.tensor.transpose(tr2, ctxn, ident_sb)
                    nc.vector.tensor_copy(
                        out=ctxT[t][r0:r0 + 64, qc * 512 + sl * 128:qc * 512 + (sl + 1) * 128],
                        in_=tr2)

        # --- output projection for all strips (cc_in in DRAM) ---
        cc_in = dram.tile([S, DM], f32, name="cc_in")
        cc_out = dram.tile([S // 2, DM], f32, name="cc_out")
        for s in range(SBLK):
            for nch in range(2):
                psum = ps_mm.tile([128, 512], f32, name=f"ops{s}_{nch}", tag="mm")
                for hb in range(OBLK):
                    nc.tensor.matmul(
                        psum, ctxT[hb][:, s * 128:(s + 1) * 128],
                        wo_sb[hb][:, nch * 512:(nch + 1) * 512],
                        start=(hb == 0), stop=False)
                nc.tensor.matmul(
                    psum, ones1, bo_sb[:, nch * 512:(nch + 1) * 512],
                    start=False, stop=True)
                osb = stg.tile([128, 512], f32, name=f"ob{s}_{nch}", tag="osb", bufs=3)
                nc.scalar.copy(out=osb, in_=psum)
                nc.sync.dma_start(
                    out=cc_in[s * 128:(s + 1) * 128, nch * 512:(nch + 1) * 512], in_=osb)

        nc.gpsimd.collective_compute(
            "ReduceScatter", mybir.AluOpType.add,
            replica_groups=[[0, 1], [2, 3], [4, 5], [6, 7]],
            ins=[cc_in.opt()], outs=[cc_out.opt()])
        nc.sync.dma_start(out=out_ext[:, :], in_=cc_out[:])

    nc.finalize()
    return nc


def _get_program():
    global _prog
    if _prog is None:
        _prog = _build_program()
    return _prog


def make_in_maps(query, key, value, Wq, bq, Wk, bk, Wv, bv, Wo, bo):
    query = np.asarray(query, dtype=np.float32)
    key = np.asarray(key, dtype=np.float32)
    value = np.asarray(value, dtype=np.float32)
    Wq = np.asarray(Wq, dtype=np.float32)
    bq = np.asarray(bq, dtype=np.float32)
    Wk = np.asarray(Wk, dtype=np.float32)
    bk = np.asarray(bk, dtype=np.float32)
    Wv = np.asarray(Wv, dtype=np.float32)
    bv = np.asarray(bv, dtype=np.float32)
    Wo = np.asarray(Wo, dtype=np.float32)
    bo = np.asarray(bo, dtype=np.float32)

    scale = 1.0 / np.sqrt(np.float32(DM))
    mask = np.triu(np.ones((128, 128), dtype=np.float32))
    ident = np.eye(128, dtype=np.float32)

    xq_t = [np.ascontiguousarray(query[b].T) for b in range(B)]
    xk_t = [np.ascontiguousarray(key[b].T) for b in range(B)]
    xv_t = [np.ascontiguousarray(value[b].T) for b in range(B)]

    in_maps = []
    for c in range(NCORES):
        b, hh = c // 2, c % 2
        cols = slice(hh * OD, (hh + 1) * OD)
        in_maps.append({
            "xq": xq_t[b],
            "xk": xk_t[b],
            "xv": xv_t[b],
            "wq": np.ascontiguousarray(Wq[:, cols] * scale),
            "wk": np.ascontiguousarray(Wk[:, cols]),
            "wv": np.ascontiguousarray(Wv[:, cols]),
            "wo": np.ascontiguousarray(Wo[cols, :]),
            "bq2": np.ascontiguousarray((bq[cols] * scale).reshape(OBLK, 128).T),
            "bk2": np.ascontiguousarray(bk[cols].reshape(OBLK, 128).T),
            "bvr": np.ascontiguousarray(bv[cols].reshape(1, OD)),
            "bo2": np.ascontiguousarray((bo / 2.0).reshape(1, DM)),
            "mask": mask,
            "ident": ident,
            "ones1": np.ones((1, 128), dtype=np.float32),
        })
    return in_maps


def kernel(query, key, value, Wq, bq, Wk, bk, Wv, bv, Wo, bo):
    from concourse.bass_utils import run_bass_kernel_spmd

    in_maps = make_in_maps(query, key, value, Wq, bq, Wk, bk, Wv, bv, Wo, bo)
    nc = _get_program()
    res = run_bass_kernel_spmd(nc, in_maps, list(range(NCORES)))

    out = np.empty((B, S, DM), dtype=np.float32)
    for b in range(B):
        out[b, : S // 2] = res.results[2 * b]["out"]
        out[b, S // 2:] = res.results[2 * b + 1]["out"]
    return out
